# revision 1
# baseline (speedup 1.0000x reference)
# Trainium2 Bass kernel for AvaAttention (GQA attention + RoPE + additive mask)
# B=2, T=2048, HID=2048, NH=16, KVH=4, HD=128, fp32 — 8 NeuronCores.
#
# Sharding: sequence-parallel. Core i (batch b=i//4, position p=i%4) owns
# q-blocks j = 4s+3-p of batch b, for slot s in 0..3. Projections are
# row-parallel (weights replicated), K/V exchanged with an AllGather over
# each batch's 4 cores, attention + output projection stay local to the
# core's rows. Compute dtype: fp32r (TF32) for projections/scores/Wo,
# bf16 for probabilities*V. Softmax uses exp(S)/sum(exp(S)) without
# max-subtraction (safe at this problem's score scale; masked positions
# hit exp(S-1e9)=0). The mask is applied as *data* via identity-matmul
# accumulation into PSUM at mask-plan positions that are uniform across
# cores, so one compiled SPMD program serves all 8 cores.

import sys

for _p in ("/opt/trn_rl_repo", "/opt/pypackages"):
    if _p not in sys.path:
        sys.path.insert(0, _p)

import numpy as np
import ml_dtypes

B, T, HID = 2, 2048, 2048
NH, KVH, HD = 16, 4, 128
P = 128
NC = 8
NBLK = T // P          # 16 q-blocks per batch
NSLOT = 4              # blocks per core
GPQ = NH // KVH        # 4 q-heads per kv group
HB = HID // P          # 16 contraction subtiles
NEG_THRESH = -1.0e8


def _mask_plan(attention_mask):
    """Classify the additive mask per (j, kb) 128x128 tile.

    Returns (E, P_list): E[s] is the uniform k-extent (in blocks) for slot
    s; P_list is the ordered list of (s, kb) positions where a mask-add is
    applied (positions shared by every core; tile *data* is per-core).
    """
    m = np.asarray(attention_mask).reshape(T, T)
    nonzero = np.zeros((NBLK, NBLK), dtype=bool)
    live = np.zeros((NBLK, NBLK), dtype=bool)   # not fully masked
    for j in range(NBLK):
        for kb in range(NBLK):
            tile = m[j * P:(j + 1) * P, kb * P:(kb + 1) * P]
            nonzero[j, kb] = bool(np.any(tile != 0.0))
            live[j, kb] = bool(np.any(tile > NEG_THRESH))
    kmax = np.ones(NBLK, dtype=int)
    for j in range(NBLK):
        idx = np.nonzero(live[j])[0]
        if len(idx):
            kmax[j] = int(idx[-1]) + 1
    E = [int(max(kmax[4 * s + jj] for jj in range(4))) for s in range(NSLOT)]
    P_list = []
    for s in range(NSLOT):
        for kb in range(E[s]):
            if any(nonzero[4 * s + jj, kb] for jj in range(4)):
                P_list.append((s, kb))
    return E, P_list


def _build_program(E, P_list):
    import concourse.mybir as mybir
    import concourse.tile as tile
    from concourse import bacc
    from concourse.masks import make_identity
    from contextlib import ExitStack

    FP32 = mybir.dt.float32
    FP32R = mybir.dt.float32r
    BF16 = mybir.dt.bfloat16
    Exp = mybir.ActivationFunctionType.Exp
    HALF = HD // 2

    nc = bacc.Bacc("TRN2", target_bir_lowering=False, num_devices=NC)

    x_p = nc.declare_dram_parameter("x", [NSLOT * P, HID], FP32, isOutput=False)
    wq_p = nc.declare_dram_parameter("wq", [HID, NH * HD], FP32, isOutput=False)
    wk_p = nc.declare_dram_parameter("wk", [HID, KVH * HD], FP32, isOutput=False)
    wv_p = nc.declare_dram_parameter("wv", [HID, KVH * HD], FP32, isOutput=False)
    wo_p = nc.declare_dram_parameter("wo", [HID, HID], FP32, isOutput=False)
    cosq_p = nc.declare_dram_parameter("cosq", [NSLOT * P, HD], FP32, isOutput=False)
    sinq_p = nc.declare_dram_parameter("sinq3", [NSLOT * P, HD], FP32, isOutput=False)
    cosk_p = nc.declare_dram_parameter("cosk", [NSLOT * P, HD], FP32, isOutput=False)
    sink_p = nc.declare_dram_parameter("sink3", [NSLOT * P, HD], FP32, isOutput=False)
    nmask = max(1, len(P_list))
    masks_p = nc.declare_dram_parameter("masks", [nmask, P, P], BF16, isOutput=False)
    out_p = nc.declare_dram_parameter("out", [NSLOT * P, HID], FP32, isOutput=True)

    KVW = KVH * HD  # 512
    ag_k_in = nc.dram_tensor("ag_k_in", [KVW, NSLOT * P], FP32)
    ag_k_out = nc.dram_tensor("ag_k_out", [4, KVW, NSLOT * P], FP32, addr_space="Local")
    ag_v_in = nc.dram_tensor("ag_v_in", [NSLOT * P, KVW], BF16)
    ag_v_out = nc.dram_tensor("ag_v_out", [4, NSLOT * P, KVW], BF16, addr_space="Local")
    groups = [[0, 1, 2, 3], [4, 5, 6, 7]]

    mask_idx = {sk: idx for idx, sk in enumerate(P_list)}

    def rope(engine, dst, src_ps, cos_t, sin_t, s, nh):
        """dst[t, h, d] = src*cos + rotate_half(src)*sin, natural layout."""
        src3 = src_ps[:].rearrange("p (h d) -> p h d", d=HD)
        cst = rope.pool.tile([P, nh, HD], FP32, name="rope_c", tag="rope_c")
        engine.tensor_tensor(dst[:], src3,
                             cos_t[:, s, None, :].to_broadcast((P, nh, HD)),
                             mybir.AluOpType.mult)
        engine.tensor_tensor(cst[:], src3,
                             sin_t[:, s, None, :].to_broadcast((P, nh, HD)),
                             mybir.AluOpType.mult)
        engine.tensor_tensor(dst[:, :, HALF:], dst[:, :, HALF:],
                             cst[:, :, :HALF], mybir.AluOpType.add)
        engine.tensor_tensor(dst[:, :, :HALF], dst[:, :, :HALF],
                             cst[:, :, HALF:], mybir.AluOpType.add)

    with tile.TileContext(nc) as tc, ExitStack() as top:
        const = top.enter_context(tc.tile_pool(name="const", bufs=1))
        ident_f32 = const.tile([P, P], FP32)
        make_identity(nc, ident_f32[:])
        ident_bf = const.tile([P, P], BF16)
        make_identity(nc, ident_bf[:])

        cosq_t = const.tile([P, NSLOT, HD], FP32)
        sinq_t = const.tile([P, NSLOT, HD], FP32)
        cosk_t = const.tile([P, NSLOT, HD], FP32)
        sink_t = const.tile([P, NSLOT, HD], FP32)
        for ap, prm in ((cosq_t, cosq_p), (sinq_t, sinq_p),
                        (cosk_t, cosk_p), (sink_t, sink_p)):
            nc.sync.dma_start(ap[:], prm[:].rearrange("(s p) d -> p s d", p=P))

        masks_t = const.tile([P, nmask, P], BF16)
        nc.sync.dma_start(masks_t[:], masks_p[:].rearrange("n p d -> p n d"))

        qT_pool = top.enter_context(tc.tile_pool(name="qT_pool", bufs=1))
        qT = qT_pool.tile([P, NH, NSLOT * P], FP32R)          # [d, h, t]

        # ================= projection phases =================
        with tc.tile_pool(name="xT_pool", bufs=1) as xT_pool, \
             tc.tile_pool(name="ph0ps", bufs=2, space="PSUM") as ps0:
            xT = xT_pool.tile([P, HB, NSLOT * P], FP32R)      # [h%128, hb, t]

            # ---- phase 0: load x, transpose to xT ----
            with tc.tile_pool(name="xph", bufs=2) as xpool:
                x_nat = []
                for s in range(NSLOT):
                    xs = xpool.tile([P, HID], FP32, name=f"x_nat{s}", tag=f"x_nat{s % 2}")
                    nc.sync.dma_start(xs[:], x_p[s * P:(s + 1) * P, :])
                    x_nat.append(xs)
                for hb in range(HB):
                    pxt = ps0.tile([P, NSLOT * P], FP32, name="pxt", tag="pxt")
                    for s in range(NSLOT):
                        nc.tensor.transpose(pxt[:, s * P:(s + 1) * P],
                                            x_nat[s][:, hb * P:(hb + 1) * P],
                                            ident_f32[:])
                    nc.vector.tensor_copy(xT[:, hb, :], pxt[:])

            # ---- phase 1a: K/V projections + RoPE + AllGather ----
            with tc.tile_pool(name="kvw", bufs=1) as kvw_pool, \
                 tc.tile_pool(name="kvstage", bufs=2) as kvstage:
                rope.pool = kvstage
                wk_sb = kvw_pool.tile([P, HB, KVW], FP32R, name="wk_sb")
                wv_sb = kvw_pool.tile([P, HB, KVW], FP32R, name="wv_sb")
                nc.sync.dma_start(wk_sb[:], wk_p[:].bitcast(FP32R)
                                  .rearrange("(hb p) n -> p hb n", p=P))
                nc.sync.dma_start(wv_sb[:], wv_p[:].bitcast(FP32R)
                                  .rearrange("(hb p) n -> p hb n", p=P))

                contrib_k = kvw_pool.tile([P, KVH, NSLOT * P], FP32, name="contrib_k")
                k_rope = []
                for s in range(NSLOT):
                    pk = ps0.tile([P, KVW], FP32, name="pk", tag="pkv")
                    for hb in range(HB):
                        nc.tensor.matmul(pk[:], xT[:, hb, s * P:(s + 1) * P],
                                         wk_sb[:, hb, :],
                                         start=(hb == 0), stop=(hb == HB - 1))
                    kr = kvw_pool.tile([P, KVH, HD], FP32, name=f"k_rope{s}")
                    rope(nc.vector, kr, pk, cosk_t, sink_t, s, KVH)
                    k_rope.append(kr)

                    pv = ps0.tile([P, KVW], FP32, name="pv", tag="pkv")
                    for hb in range(HB):
                        nc.tensor.matmul(pv[:], xT[:, hb, s * P:(s + 1) * P],
                                         wv_sb[:, hb, :],
                                         start=(hb == 0), stop=(hb == HB - 1))
                    vst = kvstage.tile([P, KVW], BF16, name=f"v_st{s}", tag="v_st")
                    nc.vector.tensor_copy(vst[:], pv[:])
                    nc.sync.dma_start(ag_v_in[s * P:(s + 1) * P, :], vst[:])

                for g in range(KVH):
                    pkt = ps0.tile([P, NSLOT * P], FP32, name="pkt", tag="pxt")
                    for s in range(NSLOT):
                        nc.tensor.transpose(pkt[:, s * P:(s + 1) * P],
                                            k_rope[s][:, g, :], ident_f32[:])
                    nc.vector.tensor_copy(contrib_k[:, g, :], pkt[:])
                nc.sync.dma_start(
                    ag_k_in[:].rearrange("(g d) t -> d g t", d=P), contrib_k[:])

                nc.gpsimd.collective_compute(
                    "AllGather", mybir.AluOpType.bypass, replica_groups=groups,
                    ins=[ag_k_in[:]], outs=[ag_k_out[:]])
                nc.gpsimd.collective_compute(
                    "AllGather", mybir.AluOpType.bypass, replica_groups=groups,
                    ins=[ag_v_in[:]], outs=[ag_v_out[:]])

            # ---- phase 1b: Q projection + RoPE + transpose to qT ----
            QC = 2  # heads per Wq chunk (SBUF pressure)
            with tc.tile_pool(name="qw", bufs=2) as qw_pool, \
                 tc.tile_pool(name="qstage", bufs=3) as qstage, \
                 tc.tile_pool(name="qtps", bufs=2, space="PSUM") as qtps:
                rope.pool = qstage
                for hc in range(NH // QC):
                    wq_sb = qw_pool.tile([P, HB, QC * HD], FP32R, name="wq_sb")
                    nc.sync.dma_start(
                        wq_sb[:],
                        wq_p[:, hc * QC * HD:(hc + 1) * QC * HD].bitcast(FP32R)
                        .rearrange("(hb p) n -> p hb n", p=P))
                    q_rope = []
                    for s in range(NSLOT):
                        pq = ps0.tile([P, QC * HD], FP32, name="pq", tag="pq")
                        for hb in range(HB):
                            nc.tensor.matmul(pq[:], xT[:, hb, s * P:(s + 1) * P],
                                             wq_sb[:, hb, :],
                                             start=(hb == 0), stop=(hb == HB - 1))
                        qr = qstage.tile([P, QC, HD], FP32, name=f"q_rope{s}",
                                         tag=f"q_rope{s % 2}")
                        rope(nc.vector, qr, pq, cosq_t, sinq_t, s, QC)
                        q_rope.append(qr)
                    for h in range(QC):
                        pqt = qtps.tile([P, NSLOT * P], FP32, name="pqt", tag="pqt")
                        for s in range(NSLOT):
                            nc.tensor.transpose(pqt[:, s * P:(s + 1) * P],
                                                q_rope[s][:, h, :], ident_f32[:])
                        nc.vector.tensor_copy(qT[:, hc * QC + h, :], pqt[:])

        # ================= gather + attention + output =================
        with tc.tile_pool(name="kv_pool", bufs=1) as kv_pool:
            kT = kv_pool.tile([P, KVH, T], FP32R)             # [d, g, t(batch)]
            v_all = kv_pool.tile([P, NBLK, KVW], BF16)        # [t%128, blk, (g d)]

            # block j was produced by in-group position pos=3-(j%4), slot s=j//4
            for j in range(NBLK):
                s, pos = j // 4, 3 - (j % 4)
                nc.sync.dma_start(v_all[:, j, :],
                                  ag_v_out[pos, s * P:(s + 1) * P, :])
                for g in range(KVH):
                    nc.sync.dma_start(
                        kT[:, g, j * P:(j + 1) * P],
                        ag_k_out[pos, g * P:(g + 1) * P,
                                 s * P:(s + 1) * P].bitcast(FP32R))

            with tc.tile_pool(name="ctxT_pool", bufs=1) as ctxT_pool:
                ctxT = ctxT_pool.tile([P, NSLOT, KVH, GPQ, P], FP32R)

                # ---- phase 3: attention ----
                CH = 512 // P
                with tc.tile_pool(name="ppool", bufs=2) as ppool, \
                     tc.tile_pool(name="astage", bufs=4) as astage, \
                     tc.tile_pool(name="dstage", bufs=2) as dstage, \
                     tc.tile_pool(name="sps", bufs=4, space="PSUM") as sps, \
                     tc.tile_pool(name="tps", bufs=2, space="PSUM") as tps, \
                     tc.tile_pool(name="cps", bufs=2, space="PSUM") as cps:
                    for s in range(NSLOT):
                        Es = E[s]
                        nch = (Es + CH - 1) // CH
                        for g in range(KVH):
                            sums = astage.tile([P, GPQ * nch], FP32,
                                               name="sums", tag="sums")
                            p_tiles = [ppool.tile([P, Es * P], BF16,
                                                  name=f"p_{h}", tag=f"p_{h}")
                                       for h in range(GPQ)]
                            for c in range(nch):
                                k0, k1 = c * CH, min(Es, (c + 1) * CH)
                                ncols = (k1 - k0) * P
                                adds = [kb for kb in range(k0, k1)
                                        if (s, kb) in mask_idx]
                                for h in range(GPQ):
                                    pss = sps.tile([P, 512], FP32,
                                                   name="pss", tag="pss")
                                    nc.tensor.matmul(
                                        pss[:, :ncols],
                                        qT[:, g * GPQ + h, s * P:(s + 1) * P],
                                        kT[:, g, k0 * P:k1 * P],
                                        start=True, stop=(not adds))
                                    for na, kb in enumerate(adds):
                                        mi = mask_idx[(s, kb)]
                                        nc.tensor.matmul(
                                            pss[:, (kb - k0) * P:(kb - k0 + 1) * P],
                                            ident_bf[:], masks_t[:, mi, :],
                                            start=False, stop=(na == len(adds) - 1))
                                    nc.scalar.activation(
                                        p_tiles[h][:, k0 * P:k1 * P],
                                        pss[:, :ncols], Exp,
                                        accum_out=sums[:, h * nch + c:
                                                       h * nch + c + 1])
                            rs = astage.tile([P, GPQ], FP32, name="rs", tag="rs")
                            nc.vector.tensor_reduce(
                                rs[:],
                                sums[:].rearrange("p (h c) -> p h c", c=nch),
                                axis=mybir.AxisListType.X,
                                op=mybir.AluOpType.add)
                            rr = astage.tile([P, GPQ], FP32, name="rr", tag="rr")
                            nc.vector.reciprocal(rr[:], rs[:])
                            diags = []
                            for h in range(GPQ):
                                dg = dstage.tile([P, P], BF16,
                                                 name=f"diag{h}", tag=f"diag{h}")
                                nc.vector.tensor_scalar_mul(dg[:], ident_bf[:],
                                                            rr[:, h:h + 1])
                                diags.append(dg)
                            pctx = cps.tile([P, GPQ * P], FP32,
                                            name="pctx", tag="pctx")
                            for kb in range(Es):
                                ppt = tps.tile([P, GPQ * P], FP32,
                                               name="ppt", tag="ppt")
                                for h in range(GPQ):
                                    nc.tensor.matmul(
                                        ppt[:, h * P:(h + 1) * P],
                                        p_tiles[h][:, kb * P:(kb + 1) * P],
                                        diags[h], start=True, stop=True)
                                pts = astage.tile([P, GPQ * P], BF16,
                                                  name="pts", tag="pts")
                                nc.vector.tensor_copy(pts[:], ppt[:])
                                nc.tensor.matmul(pctx[:],
                                                 v_all[:, kb, g * HD:(g + 1) * HD],
                                                 pts[:],
                                                 start=(kb == 0),
                                                 stop=(kb == Es - 1))
                            nc.vector.tensor_copy(
                                ctxT[:, s, g, :, :],
                                pctx[:].rearrange("p (h d) -> p h d", d=P))

                # ---- phase 4: output projection ----
                OC = 256
                with tc.tile_pool(name="wopool", bufs=2) as wopool, \
                     tc.tile_pool(name="ostage", bufs=3) as ostage, \
                     tc.tile_pool(name="ops", bufs=4, space="PSUM") as ops:
                    for oc in range(HID // OC):
                        wo_sb = wopool.tile([P, HB, OC], FP32R, name="wo_sb")
                        nc.sync.dma_start(
                            wo_sb[:],
                            wo_p[:, oc * OC:(oc + 1) * OC].bitcast(FP32R)
                            .rearrange("(hb p) n -> p hb n", p=P))
                        for s in range(NSLOT):
                            po = ops.tile([P, OC], FP32, name="po", tag="po")
                            for g in range(KVH):
                                for h in range(GPQ):
                                    hh = g * GPQ + h
                                    nc.tensor.matmul(po[:], ctxT[:, s, g, h, :],
                                                     wo_sb[:, hh, :],
                                                     start=(hh == 0),
                                                     stop=(hh == HB - 1))
                            ot = ostage.tile([P, OC], FP32, name="ot", tag="ot")
                            nc.vector.tensor_copy(ot[:], po[:])
                            nc.sync.dma_start(
                                out_p[s * P:(s + 1) * P, oc * OC:(oc + 1) * OC],
                                ot[:])

    nc.compile()
    return nc


def _prep_inputs(hidden_states, attention_mask, cos, sin, Wq, Wk, Wv, Wo, P_list):
    hs = np.ascontiguousarray(np.asarray(hidden_states, dtype=np.float32))
    mask = np.asarray(attention_mask, dtype=np.float32).reshape(T, T)
    cos2 = np.asarray(cos, dtype=np.float32).reshape(T, HD)
    sin2 = np.asarray(sin, dtype=np.float32).reshape(T, HD)
    scale = np.float32(1.0 / np.sqrt(HD))

    def t3(s_):
        # rotate_half add trick: t3 = concat(sin[:, 64:], -sin[:, :64])
        return np.concatenate([s_[:, HD // 2:], -s_[:, :HD // 2]], axis=1)

    wq = np.ascontiguousarray(np.asarray(Wq, dtype=np.float32))
    wk = np.ascontiguousarray(np.asarray(Wk, dtype=np.float32))
    wv = np.ascontiguousarray(np.asarray(Wv, dtype=np.float32))
    wo = np.ascontiguousarray(np.asarray(Wo, dtype=np.float32))

    in_maps = []
    for i in range(NC):
        b, pos = i // 4, i % 4
        js = [4 * s + 3 - pos for s in range(NSLOT)]
        take = lambda a: np.ascontiguousarray(
            np.concatenate([a[j * P:(j + 1) * P] for j in js], axis=0))
        m_tiles = [mask[js[s] * P:(js[s] + 1) * P, kb * P:(kb + 1) * P]
                   for (s, kb) in P_list]
        if not m_tiles:
            m_tiles.append(np.zeros((P, P), np.float32))
        in_maps.append({
            "x": take(hs[b]),
            "wq": wq, "wk": wk, "wv": wv, "wo": wo,
            "cosq": take(cos2 * scale),
            "sinq3": take(t3(sin2 * scale)),
            "cosk": take(cos2),
            "sink3": take(t3(sin2)),
            "masks": np.stack(m_tiles).astype(ml_dtypes.bfloat16),
        })
    return in_maps


_cache = {}


def kernel(hidden_states, attention_mask, cos, sin, Wq, Wk, Wv, Wo,
           _trace=False, _trace_kwargs=None):
    from concourse.bass_utils import run_bass_kernel_spmd

    E, P_list = _mask_plan(attention_mask)
    key = (tuple(E), tuple(P_list))
    if key not in _cache:
        _cache[key] = _build_program(E, P_list)
    nc = _cache[key]

    in_maps = _prep_inputs(hidden_states, attention_mask, cos, sin,
                           Wq, Wk, Wv, Wo, P_list)
    kwargs = dict(_trace_kwargs or {})
    if _trace:
        kwargs["trace"] = True
    res = run_bass_kernel_spmd(nc, in_maps, list(range(NC)), **kwargs)

    out = np.empty((B, T, HID), dtype=np.float32)
    for i in range(NC):
        b, pos = i // 4, i % 4
        o = res.results[i]["out"]
        for s in range(NSLOT):
            j = 4 * s + 3 - pos
            out[b, j * P:(j + 1) * P, :] = o[s * P:(s + 1) * P, :]
    kernel._last_result = res
    return out



# revision 2
# speedup vs baseline: 1.1219x; 1.1219x over previous
# Trainium2 Bass kernel for AvaAttention (GQA attention + RoPE + additive mask)
# B=2, T=2048, HID=2048, NH=16, KVH=4, HD=128, fp32 — 8 NeuronCores.
#
# Sharding: sequence-parallel. Core i (batch b=i//4, position p=i%4) owns
# q-blocks j = 4s+3-p of batch b, for slot s in 0..3. Projections are
# row-parallel (weights replicated), K/V exchanged with a SINGLE combined
# AllGather over each batch's 4 cores, attention + output projection stay
# local to the core's rows.
#
# v2 notes (vs the diag-matmul baseline):
#  - Scores are computed pre-transposed ([tk, (h tq)]) by making the K
#    block the stationary operand and streaming 4 q-heads at once
#    (N=512). This removes the 640 per-head diag/transpose matmuls and
#    the PSUM->bf16 CAST copies that dominated the vector engine.
#  - Softmax denominators come from an all-ones stationary matmul that
#    accumulates over kb next to the AV matmul; its PSUM result holds the
#    row sums replicated across all 128 partitions, so normalization is
#    one elementwise multiply fused with the PSUM->SBUF eviction of ctx.
#  - The additive mask is applied with one N=512 matmul per masked tile:
#    lhsT = mask data (natural [tq, tk]), rhs = 4 identity blocks.
#  - K and V ride ONE AllGather (flat fp32 buffer; V packed as bf16
#    pairs) instead of two serialized ones.
#  - Wo is bf16 (host-cast) and streamed in 512-column chunks.
#  - exp(S)/sum(exp(S)) without max-subtraction (safe at this score
#    scale; masked positions hit exp(S-1e9)=0).

import sys

for _p in ("/opt/trn_rl_repo", "/opt/pypackages"):
    if _p not in sys.path:
        sys.path.insert(0, _p)

import numpy as np
import ml_dtypes

B, T, HID = 2, 2048, 2048
NH, KVH, HD = 16, 4, 128
P = 128
NC = 8
NBLK = T // P          # 16 q-blocks per batch
NSLOT = 4              # blocks per core
GPQ = NH // KVH        # 4 q-heads per kv group
HB = HID // P          # 16 contraction subtiles
NEG_THRESH = -1.0e8
KVW = KVH * HD         # 512
AG_K = P * KVH * NSLOT * P       # 262144 fp32 words of transposed K
AG_V = NSLOT * P * KVW // 2      # 131072 fp32 words holding bf16 V pairs


def _mask_plan(attention_mask):
    """Classify the additive mask per (j, kb) 128x128 tile.

    Returns (E, P_list): E[s] is the uniform k-extent (in blocks) for slot
    s; P_list is the ordered list of (s, kb) positions where a mask-add is
    applied (positions shared by every core; tile *data* is per-core).
    """
    m = np.asarray(attention_mask).reshape(T, T)
    nonzero = np.zeros((NBLK, NBLK), dtype=bool)
    live = np.zeros((NBLK, NBLK), dtype=bool)   # not fully masked
    for j in range(NBLK):
        for kb in range(NBLK):
            tile = m[j * P:(j + 1) * P, kb * P:(kb + 1) * P]
            nonzero[j, kb] = bool(np.any(tile != 0.0))
            live[j, kb] = bool(np.any(tile > NEG_THRESH))
    kmax = np.ones(NBLK, dtype=int)
    for j in range(NBLK):
        idx = np.nonzero(live[j])[0]
        if len(idx):
            kmax[j] = int(idx[-1]) + 1
    E = [int(max(kmax[4 * s + jj] for jj in range(4))) for s in range(NSLOT)]
    P_list = []
    for s in range(NSLOT):
        for kb in range(E[s]):
            if any(nonzero[4 * s + jj, kb] for jj in range(4)):
                P_list.append((s, kb))
    return E, P_list


def _build_program(E, P_list):
    import concourse.mybir as mybir
    import concourse.tile as tile
    from concourse import bacc
    from concourse.masks import make_identity
    from contextlib import ExitStack

    FP32 = mybir.dt.float32
    FP32R = mybir.dt.float32r
    BF16 = mybir.dt.bfloat16
    Exp = mybir.ActivationFunctionType.Exp
    HALF = HD // 2

    nc = bacc.Bacc("TRN2", target_bir_lowering=False, num_devices=NC)

    x_p = nc.declare_dram_parameter("x", [NSLOT * P, HID], FP32, isOutput=False)
    wq_p = nc.declare_dram_parameter("wq", [HID, NH * HD], FP32, isOutput=False)
    wk_p = nc.declare_dram_parameter("wk", [HID, KVH * HD], FP32, isOutput=False)
    wv_p = nc.declare_dram_parameter("wv", [HID, KVH * HD], FP32, isOutput=False)
    wo_p = nc.declare_dram_parameter("wo", [HID, HID], BF16, isOutput=False)
    cosq_p = nc.declare_dram_parameter("cosq", [NSLOT * P, HD], FP32, isOutput=False)
    sinq_p = nc.declare_dram_parameter("sinq3", [NSLOT * P, HD], FP32, isOutput=False)
    cosk_p = nc.declare_dram_parameter("cosk", [NSLOT * P, HD], FP32, isOutput=False)
    sink_p = nc.declare_dram_parameter("sink3", [NSLOT * P, HD], FP32, isOutput=False)
    nmask = max(1, len(P_list))
    masks_p = nc.declare_dram_parameter("masks", [nmask, P, P], BF16, isOutput=False)
    out_p = nc.declare_dram_parameter("out", [NSLOT * P, HID], FP32, isOutput=True)

    ag_in = nc.dram_tensor("ag_in", [AG_K + AG_V], FP32)
    ag_out = nc.dram_tensor("ag_out", [4, AG_K + AG_V], FP32, addr_space="Local")
    groups = [[0, 1, 2, 3], [4, 5, 6, 7]]

    mask_idx = {sk: idx for idx, sk in enumerate(P_list)}

    def rope(engine, dst, src_ps, cos_t, sin_t, s, nh):
        """dst[t, h, d] = src*cos + rotate_half(src)*sin, natural layout."""
        src3 = src_ps[:].rearrange("p (h d) -> p h d", d=HD)
        cst = rope.pool.tile([P, nh, HD], FP32, name="rope_c", tag="rope_c")
        engine.tensor_tensor(dst[:], src3,
                             cos_t[:, s, None, :].to_broadcast((P, nh, HD)),
                             mybir.AluOpType.mult)
        engine.tensor_tensor(cst[:], src3,
                             sin_t[:, s, None, :].to_broadcast((P, nh, HD)),
                             mybir.AluOpType.mult)
        engine.tensor_tensor(dst[:, :, HALF:], dst[:, :, HALF:],
                             cst[:, :, :HALF], mybir.AluOpType.add)
        engine.tensor_tensor(dst[:, :, :HALF], dst[:, :, :HALF],
                             cst[:, :, HALF:], mybir.AluOpType.add)

    with tile.TileContext(nc) as tc, ExitStack() as top:
        const = top.enter_context(tc.tile_pool(name="const", bufs=1))
        ident_f32 = const.tile([P, P], FP32)
        make_identity(nc, ident_f32[:])
        # 4 identity blocks side by side: rhs of the mask-broadcast matmul
        ident4_bf = const.tile([P, GPQ, P], BF16)
        for h in range(GPQ):
            make_identity(nc, ident4_bf[:, h, :])
        ones_bf = const.tile([P, P], BF16)
        nc.gpsimd.memset(ones_bf[:], 1.0)

        cosq_t = const.tile([P, NSLOT, HD], FP32)
        sinq_t = const.tile([P, NSLOT, HD], FP32)
        cosk_t = const.tile([P, NSLOT, HD], FP32)
        sink_t = const.tile([P, NSLOT, HD], FP32)
        for ap, prm in ((cosq_t, cosq_p), (sinq_t, sinq_p),
                        (cosk_t, cosk_p), (sink_t, sink_p)):
            nc.sync.dma_start(ap[:], prm[:].rearrange("(s p) d -> p s d", p=P))

        masks_t = const.tile([P, nmask, P], BF16)
        nc.sync.dma_start(masks_t[:], masks_p[:].rearrange("n p d -> p n d"))

        qT_pool = top.enter_context(tc.tile_pool(name="qT_pool", bufs=1))
        qT = qT_pool.tile([P, NH, NSLOT * P], FP32R)          # [d, h, t]

        # ================= projection phases =================
        with tc.tile_pool(name="xT_pool", bufs=1) as xT_pool, \
             tc.tile_pool(name="ph0ps", bufs=2, space="PSUM") as ps0:
            xT = xT_pool.tile([P, HB, NSLOT * P], FP32R)      # [h%128, hb, t]

            # ---- phase 0: load x, transpose to xT ----
            with tc.tile_pool(name="xph", bufs=2) as xpool:
                x_nat = []
                for s in range(NSLOT):
                    xs = xpool.tile([P, HID], FP32, name=f"x_nat{s}", tag=f"x_nat{s % 2}")
                    nc.sync.dma_start(xs[:], x_p[s * P:(s + 1) * P, :])
                    x_nat.append(xs)
                for hb in range(HB):
                    pxt = ps0.tile([P, NSLOT * P], FP32, name="pxt", tag="pxt")
                    for s in range(NSLOT):
                        nc.tensor.transpose(pxt[:, s * P:(s + 1) * P],
                                            x_nat[s][:, hb * P:(hb + 1) * P],
                                            ident_f32[:])
                    nc.vector.tensor_copy(xT[:, hb, :], pxt[:])

            # ---- phase 1a: K/V projections + RoPE + combined AllGather ----
            with tc.tile_pool(name="kvw", bufs=1) as kvw_pool, \
                 tc.tile_pool(name="kvstage", bufs=2) as kvstage:
                rope.pool = kvstage
                wk_sb = kvw_pool.tile([P, HB, KVW], FP32R, name="wk_sb")
                wv_sb = kvw_pool.tile([P, HB, KVW], FP32R, name="wv_sb")
                nc.sync.dma_start(wk_sb[:], wk_p[:].bitcast(FP32R)
                                  .rearrange("(hb p) n -> p hb n", p=P))
                nc.sync.dma_start(wv_sb[:], wv_p[:].bitcast(FP32R)
                                  .rearrange("(hb p) n -> p hb n", p=P))

                contrib_k = kvw_pool.tile([P, KVH, NSLOT * P], FP32, name="contrib_k")
                k_rope = []
                for s in range(NSLOT):
                    pk = ps0.tile([P, KVW], FP32, name="pk", tag="pkv")
                    for hb in range(HB):
                        nc.tensor.matmul(pk[:], xT[:, hb, s * P:(s + 1) * P],
                                         wk_sb[:, hb, :],
                                         start=(hb == 0), stop=(hb == HB - 1))
                    kr = kvw_pool.tile([P, KVH, HD], FP32, name=f"k_rope{s}")
                    rope(nc.vector, kr, pk, cosk_t, sink_t, s, KVH)
                    k_rope.append(kr)

                    pv = ps0.tile([P, KVW], FP32, name="pv", tag="pkv")
                    for hb in range(HB):
                        nc.tensor.matmul(pv[:], xT[:, hb, s * P:(s + 1) * P],
                                         wv_sb[:, hb, :],
                                         start=(hb == 0), stop=(hb == HB - 1))
                    vst = kvstage.tile([P, KVW], BF16, name=f"v_st{s}", tag="v_st")
                    nc.vector.tensor_copy(vst[:], pv[:])
                    nc.sync.dma_start(
                        ag_in[AG_K + s * P * KVW // 2:
                              AG_K + (s + 1) * P * KVW // 2]
                        .rearrange("(p w) -> p w", p=P),
                        vst[:].bitcast(FP32))

                for g in range(KVH):
                    pkt = ps0.tile([P, NSLOT * P], FP32, name="pkt", tag="pxt")
                    for s in range(NSLOT):
                        nc.tensor.transpose(pkt[:, s * P:(s + 1) * P],
                                            k_rope[s][:, g, :], ident_f32[:])
                    nc.vector.tensor_copy(contrib_k[:, g, :], pkt[:])
                nc.sync.dma_start(
                    ag_in[0:AG_K].rearrange("(d g t) -> d g t", d=P, g=KVH),
                    contrib_k[:])

                nc.gpsimd.collective_compute(
                    "AllGather", mybir.AluOpType.bypass, replica_groups=groups,
                    ins=[ag_in[:]], outs=[ag_out[:]])

            # ---- phase 1b: Q projection + RoPE + transpose to qT ----
            QC = 4  # heads per Wq chunk
            with tc.tile_pool(name="qw", bufs=2) as qw_pool, \
                 tc.tile_pool(name="qstage", bufs=3) as qstage, \
                 tc.tile_pool(name="qps", bufs=2, space="PSUM") as qps, \
                 tc.tile_pool(name="qtps", bufs=2, space="PSUM") as qtps:
                rope.pool = qstage
                for hc in range(NH // QC):
                    wq_sb = qw_pool.tile([P, HB, QC * HD], FP32R, name="wq_sb")
                    nc.sync.dma_start(
                        wq_sb[:],
                        wq_p[:, hc * QC * HD:(hc + 1) * QC * HD].bitcast(FP32R)
                        .rearrange("(hb p) n -> p hb n", p=P))
                    q_rope = []
                    for s in range(NSLOT):
                        pq = qps.tile([P, QC * HD], FP32, name="pq", tag="pq")
                        for hb in range(HB):
                            nc.tensor.matmul(pq[:], xT[:, hb, s * P:(s + 1) * P],
                                             wq_sb[:, hb, :],
                                             start=(hb == 0), stop=(hb == HB - 1))
                        qr = qstage.tile([P, QC, HD], FP32, name=f"q_rope{s}",
                                         tag=f"q_rope{s % 2}")
                        rope(nc.vector, qr, pq, cosq_t, sinq_t, s, QC)
                        q_rope.append(qr)
                    for h in range(QC):
                        pqt = qtps.tile([P, NSLOT * P], FP32, name="pqt", tag="pqt")
                        for s in range(NSLOT):
                            nc.tensor.transpose(pqt[:, s * P:(s + 1) * P],
                                                q_rope[s][:, h, :], ident_f32[:])
                        nc.vector.tensor_copy(qT[:, hc * QC + h, :], pqt[:])

        # ================= gather + attention + output =================
        with tc.tile_pool(name="ctxT_pool", bufs=1) as ctxT_pool:
            ctxT = ctxT_pool.tile([P, NSLOT, KVH, GPQ, P], BF16)

            with tc.tile_pool(name="kv_pool", bufs=1) as kv_pool:
                kT = kv_pool.tile([P, KVH, T], FP32R)         # [d, g, t(batch)]
                v_all = kv_pool.tile([P, NBLK, KVW], BF16)    # [t%128, blk, (g d)]

                # block j was produced by in-group position pos=3-(j%4), slot s=j//4
                for j in range(NBLK):
                    s, pos = j // 4, 3 - (j % 4)
                    nc.sync.dma_start(
                        v_all[:, j, :],
                        ag_out[pos, AG_K + s * P * KVW // 2:
                               AG_K + (s + 1) * P * KVW // 2]
                        .rearrange("(p w) -> p w", p=P).bitcast(BF16))
                    nc.sync.dma_start(
                        kT[:, :, j * P:(j + 1) * P],
                        ag_out[pos, 0:AG_K]
                        .rearrange("(d g t) -> d g t", d=P, g=KVH)
                        [:, :, s * P:(s + 1) * P].bitcast(FP32R))

                # ---- phase 3: attention, scores computed pre-transposed ----
                with tc.tile_pool(name="ppool", bufs=3) as ppool, \
                     tc.tile_pool(name="astage", bufs=2) as astage, \
                     tc.tile_pool(name="scps", bufs=3, space="PSUM") as scps, \
                     tc.tile_pool(name="cps", bufs=2, space="PSUM") as cps, \
                     tc.tile_pool(name="rps", bufs=2, space="PSUM") as rps:
                    for s in range(NSLOT):
                        Es = E[s]
                        for g in range(KVH):
                            q_rhs = qT[:, g * GPQ:(g + 1) * GPQ, s * P:(s + 1) * P]
                            pctx = cps.tile([P, GPQ * P], FP32,
                                            name="pctx", tag="pctx")
                            prs = rps.tile([P, GPQ * P], FP32,
                                           name="prs", tag="prs")
                            for kb in range(Es):
                                psc = scps.tile([P, GPQ * P], FP32,
                                                name="psc", tag="psc")
                                mi = mask_idx.get((s, kb))
                                nc.tensor.matmul(
                                    psc[:], kT[:, g, kb * P:(kb + 1) * P],
                                    q_rhs, start=True, stop=(mi is None))
                                if mi is not None:
                                    nc.tensor.matmul(
                                        psc[:], masks_t[:, mi, :], ident4_bf[:],
                                        start=False, stop=True)
                                pt = ppool.tile([P, GPQ * P], BF16,
                                                name="pt", tag="pt")
                                nc.scalar.activation(pt[:], psc[:], Exp)
                                nc.tensor.matmul(pctx[:],
                                                 v_all[:, kb, g * HD:(g + 1) * HD],
                                                 pt[:],
                                                 start=(kb == 0),
                                                 stop=(kb == Es - 1))
                                nc.tensor.matmul(prs[:], ones_bf[:], pt[:],
                                                 start=(kb == 0),
                                                 stop=(kb == Es - 1))
                            rr = astage.tile([P, GPQ * P], FP32,
                                             name="rr", tag="rr")
                            nc.vector.reciprocal(rr[:], prs[:])
                            nc.vector.tensor_tensor(
                                ctxT[:, s, g, :, :],
                                pctx[:].rearrange("p (h t) -> p h t", t=P),
                                rr[:].rearrange("p (h t) -> p h t", t=P),
                                mybir.AluOpType.mult)

            # ---- phase 4: output projection (Wo in bf16, 512-col chunks) ----
            OC = 512
            with tc.tile_pool(name="wopool", bufs=2) as wopool, \
                 tc.tile_pool(name="ostage", bufs=3) as ostage, \
                 tc.tile_pool(name="ops", bufs=2, space="PSUM") as ops:
                for oc in range(HID // OC):
                    wo_sb = wopool.tile([P, HB, OC], BF16, name="wo_sb")
                    nc.sync.dma_start(
                        wo_sb[:],
                        wo_p[:, oc * OC:(oc + 1) * OC]
                        .rearrange("(hb p) n -> p hb n", p=P))
                    for s in range(NSLOT):
                        po = ops.tile([P, OC], FP32, name="po", tag="po")
                        for g in range(KVH):
                            for h in range(GPQ):
                                hh = g * GPQ + h
                                nc.tensor.matmul(po[:], ctxT[:, s, g, h, :],
                                                 wo_sb[:, hh, :],
                                                 start=(hh == 0),
                                                 stop=(hh == HB - 1))
                        ot = ostage.tile([P, OC], FP32, name="ot", tag="ot")
                        nc.vector.tensor_copy(ot[:], po[:])
                        nc.sync.dma_start(
                            out_p[s * P:(s + 1) * P, oc * OC:(oc + 1) * OC],
                            ot[:])

    nc.compile()
    return nc


def _prep_inputs(hidden_states, attention_mask, cos, sin, Wq, Wk, Wv, Wo, P_list):
    hs = np.ascontiguousarray(np.asarray(hidden_states, dtype=np.float32))
    mask = np.asarray(attention_mask, dtype=np.float32).reshape(T, T)
    cos2 = np.asarray(cos, dtype=np.float32).reshape(T, HD)
    sin2 = np.asarray(sin, dtype=np.float32).reshape(T, HD)
    scale = np.float32(1.0 / np.sqrt(HD))

    def t3(s_):
        # rotate_half add trick: t3 = concat(sin[:, 64:], -sin[:, :64])
        return np.concatenate([s_[:, HD // 2:], -s_[:, :HD // 2]], axis=1)

    wq = np.ascontiguousarray(np.asarray(Wq, dtype=np.float32))
    wk = np.ascontiguousarray(np.asarray(Wk, dtype=np.float32))
    wv = np.ascontiguousarray(np.asarray(Wv, dtype=np.float32))
    wo = np.ascontiguousarray(
        np.asarray(Wo, dtype=np.float32).astype(ml_dtypes.bfloat16))

    in_maps = []
    for i in range(NC):
        b, pos = i // 4, i % 4
        js = [4 * s + 3 - pos for s in range(NSLOT)]
        take = lambda a: np.ascontiguousarray(
            np.concatenate([a[j * P:(j + 1) * P] for j in js], axis=0))
        m_tiles = [mask[js[s] * P:(js[s] + 1) * P, kb * P:(kb + 1) * P]
                   for (s, kb) in P_list]
        if not m_tiles:
            m_tiles.append(np.zeros((P, P), np.float32))
        in_maps.append({
            "x": take(hs[b]),
            "wq": wq, "wk": wk, "wv": wv, "wo": wo,
            "cosq": take(cos2 * scale),
            "sinq3": take(t3(sin2 * scale)),
            "cosk": take(cos2),
            "sink3": take(t3(sin2)),
            "masks": np.stack(m_tiles).astype(ml_dtypes.bfloat16),
        })
    return in_maps


_cache = {}


def kernel(hidden_states, attention_mask, cos, sin, Wq, Wk, Wv, Wo,
           _trace=False, _trace_kwargs=None):
    from concourse.bass_utils import run_bass_kernel_spmd

    E, P_list = _mask_plan(attention_mask)
    key = (tuple(E), tuple(P_list))
    if key not in _cache:
        _cache[key] = _build_program(E, P_list)
    nc = _cache[key]

    in_maps = _prep_inputs(hidden_states, attention_mask, cos, sin,
                           Wq, Wk, Wv, Wo, P_list)
    kwargs = dict(_trace_kwargs or {})
    if _trace:
        kwargs["trace"] = True
    res = run_bass_kernel_spmd(nc, in_maps, list(range(NC)), **kwargs)

    out = np.empty((B, T, HID), dtype=np.float32)
    for i in range(NC):
        b, pos = i // 4, i % 4
        o = res.results[i]["out"]
        for s in range(NSLOT):
            j = 4 * s + 3 - pos
            out[b, j * P:(j + 1) * P, :] = o[s * P:(s + 1) * P, :]
    kernel._last_result = res
    return out


# revision 4
# speedup vs baseline: 1.2127x; 1.0809x over previous
# Trainium2 Bass kernel for AvaAttention (GQA attention + RoPE + additive mask)
# B=2, T=2048, HID=2048, NH=16, KVH=4, HD=128, fp32 — 8 NeuronCores.
#
# Sharding: sequence-parallel. Core i (batch b=i//4, position p=i%4) owns
# q-blocks j = 4s+3-p of batch b, for slot s in 0..3. Projections are
# row-parallel (weights replicated, bf16), K/V exchanged with a SINGLE
# combined AllGather (bf16 payloads packed in a flat fp32 buffer) over
# each batch's 4 cores; attention + output projection stay local.
#
# v3 notes:
#  - Projections run in bf16 (bf16 xT + bf16 weights); RoPE in fp32 from
#    the fp32 PSUM projection result; q/k re-cast to bf16 on the
#    PSUM->SBUF eviction after their transposes.
#  - Scores are computed pre-transposed ([tk, (h tq)]): K block is the
#    stationary operand, 4 q-heads stream at once (N=512). No per-head
#    diag/transpose matmuls, no PSUM->bf16 CAST of probabilities.
#  - Softmax denominators via an all-ones stationary matmul accumulated
#    over kb; result is replicated across partitions so normalization is
#    one elementwise multiply fused into the ctx PSUM->SBUF eviction.
#    1/x via the fast custom-DVE reciprocal (plain reciprocal is ~3.4us
#    per tile and serialized the attention tail).
#  - Additive mask: one N=512 matmul per masked tile (lhsT = mask data in
#    natural [tq, tk], rhs = 4 identity blocks).
#  - Wo is bf16, fully resident in SBUF, and its matmuls interleave with
#    attention per-slot so the tensor engine stays busy to the end.
#  - exp without max-subtraction (safe at this score scale; masked
#    positions hit exp(S-1e9)=0).

import sys

for _p in ("/opt/trn_rl_repo", "/opt/pypackages"):
    if _p not in sys.path:
        sys.path.insert(0, _p)

import numpy as np
import ml_dtypes

B, T, HID = 2, 2048, 2048
NH, KVH, HD = 16, 4, 128
P = 128
NC = 8
NBLK = T // P          # 16 q-blocks per batch
NSLOT = 4              # blocks per core
GPQ = NH // KVH        # 4 q-heads per kv group
HB = HID // P          # 16 contraction subtiles
NEG_THRESH = -1.0e8
KVW = KVH * HD         # 512
SLOT_K = P * KVH * P // 2        # 32768 fp32 words: one slot's bf16 kT
SLOT_V = P * KVW // 2            # 32768 fp32 words: one slot's bf16 V
AG_K = NSLOT * SLOT_K            # 131072
AG_V = NSLOT * SLOT_V            # 131072


def _mask_plan(attention_mask):
    """Classify the additive mask per (j, kb) 128x128 tile.

    Returns (E, P_list): E[s] is the uniform k-extent (in blocks) for slot
    s; P_list is the ordered list of (s, kb) positions where a mask-add is
    applied (positions shared by every core; tile *data* is per-core).
    """
    m = np.asarray(attention_mask).reshape(T, T)
    nonzero = np.zeros((NBLK, NBLK), dtype=bool)
    live = np.zeros((NBLK, NBLK), dtype=bool)   # not fully masked
    for j in range(NBLK):
        for kb in range(NBLK):
            tile = m[j * P:(j + 1) * P, kb * P:(kb + 1) * P]
            nonzero[j, kb] = bool(np.any(tile != 0.0))
            live[j, kb] = bool(np.any(tile > NEG_THRESH))
    kmax = np.ones(NBLK, dtype=int)
    for j in range(NBLK):
        idx = np.nonzero(live[j])[0]
        if len(idx):
            kmax[j] = int(idx[-1]) + 1
    E = [int(max(kmax[4 * s + jj] for jj in range(4))) for s in range(NSLOT)]
    P_list = []
    for s in range(NSLOT):
        for kb in range(E[s]):
            if any(nonzero[4 * s + jj, kb] for jj in range(4)):
                P_list.append((s, kb))
    return E, P_list


def _build_program(E, P_list):
    import concourse.mybir as mybir
    import concourse.tile as tile
    from concourse import bacc
    from concourse.masks import make_identity
    from contextlib import ExitStack

    FP32 = mybir.dt.float32
    FP32R = mybir.dt.float32r
    BF16 = mybir.dt.bfloat16
    Exp = mybir.ActivationFunctionType.Exp
    HALF = HD // 2

    nc = bacc.Bacc("TRN2", target_bir_lowering=False, num_devices=NC)

    x_p = nc.declare_dram_parameter("x", [NSLOT * P, HID], FP32, isOutput=False)
    wq_p = nc.declare_dram_parameter("wq", [HID, NH * HD], BF16, isOutput=False)
    wk_p = nc.declare_dram_parameter("wk", [HID, KVH * HD], BF16, isOutput=False)
    wv_p = nc.declare_dram_parameter("wv", [HID, KVH * HD], BF16, isOutput=False)
    wo_p = nc.declare_dram_parameter("wo", [HID, HID], BF16, isOutput=False)
    cosq_p = nc.declare_dram_parameter("cosq", [NSLOT * P, HD], FP32, isOutput=False)
    sinq_p = nc.declare_dram_parameter("sinq3", [NSLOT * P, HD], FP32, isOutput=False)
    cosk_p = nc.declare_dram_parameter("cosk", [NSLOT * P, HD], FP32, isOutput=False)
    sink_p = nc.declare_dram_parameter("sink3", [NSLOT * P, HD], FP32, isOutput=False)
    nmask = max(1, len(P_list))
    masks_p = nc.declare_dram_parameter("masks", [nmask, P, P], BF16, isOutput=False)
    out_p = nc.declare_dram_parameter("out", [NSLOT * P, HID], FP32, isOutput=True)

    ag_in = nc.dram_tensor("ag_in", [AG_K + AG_V], FP32)
    ag_out = nc.dram_tensor("ag_out", [4, AG_K + AG_V], FP32, addr_space="Local")
    groups = [[0, 1, 2, 3], [4, 5, 6, 7]]

    mask_idx = {sk: idx for idx, sk in enumerate(P_list)}

    def rope(engine, dst, src_ps, cos_t, sin_t, s, nh):
        """dst[t, h, d] = src*cos + rotate_half(src)*sin, natural layout."""
        src3 = src_ps[:].rearrange("p (h d) -> p h d", d=HD)
        cst = rope.pool.tile([P, nh, HD], FP32, name="rope_c", tag="rope_c")
        engine.tensor_tensor(dst[:], src3,
                             cos_t[:, s, None, :].to_broadcast((P, nh, HD)),
                             mybir.AluOpType.mult)
        engine.tensor_tensor(cst[:], src3,
                             sin_t[:, s, None, :].to_broadcast((P, nh, HD)),
                             mybir.AluOpType.mult)
        engine.tensor_tensor(dst[:, :, HALF:], dst[:, :, HALF:],
                             cst[:, :, :HALF], mybir.AluOpType.add)
        engine.tensor_tensor(dst[:, :, :HALF], dst[:, :, :HALF],
                             cst[:, :, HALF:], mybir.AluOpType.add)

    with tile.TileContext(nc) as tc, ExitStack() as top:
        const = top.enter_context(tc.tile_pool(name="const", bufs=1))
        ident_f32 = const.tile([P, P], FP32)
        make_identity(nc, ident_f32[:])
        # 4 identity blocks side by side: rhs of the mask-broadcast matmul
        ident4_bf = const.tile([P, GPQ, P], BF16)
        for h in range(GPQ):
            make_identity(nc, ident4_bf[:, h, :])
        ones_bf = const.tile([P, P], BF16)
        nc.gpsimd.memset(ones_bf[:], 1.0)

        cosq_t = const.tile([P, NSLOT, HD], FP32)
        sinq_t = const.tile([P, NSLOT, HD], FP32)
        cosk_t = const.tile([P, NSLOT, HD], FP32)
        sink_t = const.tile([P, NSLOT, HD], FP32)
        for ap, prm in ((cosq_t, cosq_p), (sinq_t, sinq_p),
                        (cosk_t, cosk_p), (sink_t, sink_p)):
            nc.sync.dma_start(ap[:], prm[:].rearrange("(s p) d -> p s d", p=P))

        masks_t = const.tile([P, nmask, P], BF16)
        nc.sync.dma_start(masks_t[:], masks_p[:].rearrange("n p d -> p n d"))

        qT_pool = top.enter_context(tc.tile_pool(name="qT_pool", bufs=1))
        qT = qT_pool.tile([P, NH, NSLOT * P], BF16)           # [d, h, t]

        # ================= projection phases =================
        with tc.tile_pool(name="xT_pool", bufs=1) as xT_pool, \
             tc.tile_pool(name="ph0ps", bufs=2, space="PSUM") as ps0:
            xT = xT_pool.tile([P, HB, NSLOT * P], BF16)       # [h%128, hb, t]

            # ---- phase 0: load x, transpose to xT (bf16 on eviction) ----
            with tc.tile_pool(name="xph", bufs=2) as xpool:
                x_nat = []
                for s in range(NSLOT):
                    xs = xpool.tile([P, HID], FP32, name=f"x_nat{s}", tag=f"x_nat{s % 2}")
                    nc.sync.dma_start(xs[:], x_p[s * P:(s + 1) * P, :])
                    x_nat.append(xs)
                for hb in range(HB):
                    pxt = ps0.tile([P, NSLOT * P], FP32, name="pxt", tag="pxt")
                    for s in range(NSLOT):
                        nc.tensor.transpose(pxt[:, s * P:(s + 1) * P],
                                            x_nat[s][:, hb * P:(hb + 1) * P],
                                            ident_f32[:])
                    nc.vector.tensor_copy(xT[:, hb, :], pxt[:])

            # ---- phase 1a: K/V proj + RoPE + per-slot staging + AllGather ----
            with tc.tile_pool(name="kvw", bufs=1) as kvw_pool, \
                 tc.tile_pool(name="kvstage", bufs=2) as kvstage, \
                 tc.tile_pool(name="ktps", bufs=2, space="PSUM") as ktps:
                rope.pool = kvstage
                wk_sb = kvw_pool.tile([P, HB, KVW], BF16, name="wk_sb")
                wv_sb = kvw_pool.tile([P, HB, KVW], BF16, name="wv_sb")
                nc.sync.dma_start(wk_sb[:],
                                  wk_p[:].rearrange("(hb p) n -> p hb n", p=P))
                nc.sync.dma_start(wv_sb[:],
                                  wv_p[:].rearrange("(hb p) n -> p hb n", p=P))

                for s in range(NSLOT):
                    pk = ps0.tile([P, KVW], FP32, name="pk", tag="pkv")
                    for hb in range(HB):
                        nc.tensor.matmul(pk[:], xT[:, hb, s * P:(s + 1) * P],
                                         wk_sb[:, hb, :],
                                         start=(hb == 0), stop=(hb == HB - 1))
                    kr = kvstage.tile([P, KVH, HD], FP32, name=f"k_rope{s}",
                                      tag=f"k_rope{s % 2}")
                    rope(nc.vector, kr, pk, cosk_t, sink_t, s, KVH)

                    pv = ps0.tile([P, KVW], FP32, name="pv", tag="pkv")
                    for hb in range(HB):
                        nc.tensor.matmul(pv[:], xT[:, hb, s * P:(s + 1) * P],
                                         wv_sb[:, hb, :],
                                         start=(hb == 0), stop=(hb == HB - 1))
                    vst = kvstage.tile([P, KVW], BF16, name=f"v_st{s}", tag="v_st")
                    nc.vector.tensor_copy(vst[:], pv[:])
                    nc.sync.dma_start(
                        ag_in[AG_K + s * SLOT_V:AG_K + (s + 1) * SLOT_V]
                        .rearrange("(p w) -> p w", p=P),
                        vst[:].bitcast(FP32))

                    # transpose this slot's k and stage it (bf16)
                    pkt = ktps.tile([P, KVH * P], FP32, name="pkt", tag="pkt")
                    for g in range(KVH):
                        nc.tensor.transpose(pkt[:, g * P:(g + 1) * P],
                                            kr[:, g, :], ident_f32[:])
                    kst = kvstage.tile([P, KVH, P], BF16, name=f"k_st{s}",
                                       tag="k_st")
                    nc.vector.tensor_copy(
                        kst[:], pkt[:].rearrange("p (g t) -> p g t", t=P))
                    nc.sync.dma_start(
                        ag_in[s * SLOT_K:(s + 1) * SLOT_K]
                        .rearrange("(d g w) -> d g w", d=P, g=KVH),
                        kst[:].bitcast(FP32))

                nc.gpsimd.collective_compute(
                    "AllGather", mybir.AluOpType.bypass, replica_groups=groups,
                    ins=[ag_in[:]], outs=[ag_out[:]])

            # ---- phase 1b: Q projection + RoPE + transpose to qT ----
            QC = 4  # heads per Wq chunk
            with tc.tile_pool(name="qw", bufs=3) as qw_pool, \
                 tc.tile_pool(name="qstage", bufs=3) as qstage, \
                 tc.tile_pool(name="qps", bufs=2, space="PSUM") as qps, \
                 tc.tile_pool(name="qtps", bufs=2, space="PSUM") as qtps:
                rope.pool = qstage
                for hc in range(NH // QC):
                    wq_sb = qw_pool.tile([P, HB, QC * HD], BF16, name="wq_sb")
                    nc.sync.dma_start(
                        wq_sb[:],
                        wq_p[:, hc * QC * HD:(hc + 1) * QC * HD]
                        .rearrange("(hb p) n -> p hb n", p=P))
                    q_rope = []
                    for s in range(NSLOT):
                        pq = qps.tile([P, QC * HD], FP32, name="pq", tag="pq")
                        for hb in range(HB):
                            nc.tensor.matmul(pq[:], xT[:, hb, s * P:(s + 1) * P],
                                             wq_sb[:, hb, :],
                                             start=(hb == 0), stop=(hb == HB - 1))
                        qr = qstage.tile([P, QC, HD], FP32, name=f"q_rope{s}",
                                         tag=f"q_rope{s % 2}")
                        rope(nc.vector, qr, pq, cosq_t, sinq_t, s, QC)
                        q_rope.append(qr)
                    for h in range(QC):
                        pqt = qtps.tile([P, NSLOT * P], FP32, name="pqt", tag="pqt")
                        for s in range(NSLOT):
                            nc.tensor.transpose(pqt[:, s * P:(s + 1) * P],
                                                q_rope[s][:, h, :], ident_f32[:])
                        nc.vector.tensor_copy(qT[:, hc * QC + h, :], pqt[:])

        # ================= gather + attention + interleaved Wo =================
        with tc.tile_pool(name="kv_pool", bufs=1) as kv_pool, \
             tc.tile_pool(name="wopool", bufs=1) as wopool:
            kT = kv_pool.tile([P, KVH, T], BF16)          # [d, g, t(batch)]
            v_all = kv_pool.tile([P, NBLK, KVW], BF16)    # [t%128, blk, (g d)]

            # block j was produced by in-group position pos=3-(j%4), slot s=j//4
            for j in range(NBLK):
                s, pos = j // 4, 3 - (j % 4)
                nc.sync.dma_start(
                    v_all[:, j, :],
                    ag_out[pos, AG_K + s * SLOT_V:AG_K + (s + 1) * SLOT_V]
                    .rearrange("(p w) -> p w", p=P).bitcast(BF16))
                nc.sync.dma_start(
                    kT[:, :, j * P:(j + 1) * P],
                    ag_out[pos, s * SLOT_K:(s + 1) * SLOT_K]
                    .rearrange("(d g w) -> d g w", d=P, g=KVH).bitcast(BF16))

            wo_sb = wopool.tile([P, HB, HID], BF16, name="wo_sb")
            nc.sync.dma_start(wo_sb[:],
                              wo_p[:].rearrange("(hb p) n -> p hb n", p=P))

            with tc.tile_pool(name="ppool", bufs=3) as ppool, \
                 tc.tile_pool(name="astage", bufs=2) as astage, \
                 tc.tile_pool(name="ctxp", bufs=2) as ctxp, \
                 tc.tile_pool(name="ostage", bufs=3) as ostage, \
                 tc.tile_pool(name="scps", bufs=2, space="PSUM") as scps, \
                 tc.tile_pool(name="cps", bufs=2, space="PSUM") as cps, \
                 tc.tile_pool(name="rps", bufs=2, space="PSUM") as rps, \
                 tc.tile_pool(name="ops", bufs=2, space="PSUM") as ops:
                for s in range(NSLOT):
                    Es = E[s]
                    ctx_s = ctxp.tile([P, KVH, GPQ, P], BF16, name=f"ctx{s}",
                                      tag=f"ctx{s % 2}")
                    for g in range(KVH):
                        q_rhs = qT[:, g * GPQ:(g + 1) * GPQ, s * P:(s + 1) * P]
                        pctx = cps.tile([P, GPQ * P], FP32, name="pctx", tag="pctx")
                        prs = rps.tile([P, GPQ * P], FP32, name="prs", tag="prs")
                        for kb in range(Es):
                            psc = scps.tile([P, GPQ * P], FP32, name="psc", tag="psc")
                            mi = mask_idx.get((s, kb))
                            nc.tensor.matmul(
                                psc[:], kT[:, g, kb * P:(kb + 1) * P],
                                q_rhs, start=True, stop=(mi is None))
                            if mi is not None:
                                nc.tensor.matmul(
                                    psc[:], masks_t[:, mi, :], ident4_bf[:],
                                    start=False, stop=True)
                            pt = ppool.tile([P, GPQ * P], BF16, name="pt", tag="pt")
                            nc.scalar.activation(pt[:], psc[:], Exp)
                            nc.tensor.matmul(pctx[:],
                                             v_all[:, kb, g * HD:(g + 1) * HD],
                                             pt[:],
                                             start=(kb == 0), stop=(kb == Es - 1))
                            nc.tensor.matmul(prs[:], ones_bf[:], pt[:],
                                             start=(kb == 0), stop=(kb == Es - 1))
                        rr = astage.tile([P, GPQ * P], FP32, name="rr", tag="rr")
                        nc.vector.reciprocal_approx_fast(rr[:], prs[:])
                        nc.vector.tensor_tensor(
                            ctx_s[:, g, :, :],
                            pctx[:].rearrange("p (h t) -> p h t", t=P),
                            rr[:].rearrange("p (h t) -> p h t", t=P),
                            mybir.AluOpType.mult)

                    # ---- Wo for this slot ----
                    OC = 512
                    for oc in range(HID // OC):
                        po = ops.tile([P, OC], FP32, name="po", tag="po")
                        for g in range(KVH):
                            for h in range(GPQ):
                                hh = g * GPQ + h
                                nc.tensor.matmul(po[:], ctx_s[:, g, h, :],
                                                 wo_sb[:, hh, oc * OC:(oc + 1) * OC],
                                                 start=(hh == 0),
                                                 stop=(hh == HB - 1))
                        ot = ostage.tile([P, OC], FP32, name="ot", tag="ot")
                        nc.vector.tensor_copy(ot[:], po[:])
                        nc.sync.dma_start(
                            out_p[s * P:(s + 1) * P, oc * OC:(oc + 1) * OC],
                            ot[:])

    nc.compile()
    return nc


def _prep_inputs(hidden_states, attention_mask, cos, sin, Wq, Wk, Wv, Wo, P_list):
    hs = np.ascontiguousarray(np.asarray(hidden_states, dtype=np.float32))
    mask = np.asarray(attention_mask, dtype=np.float32).reshape(T, T)
    cos2 = np.asarray(cos, dtype=np.float32).reshape(T, HD)
    sin2 = np.asarray(sin, dtype=np.float32).reshape(T, HD)
    scale = np.float32(1.0 / np.sqrt(HD))

    def t3(s_):
        # rotate_half add trick: t3 = concat(sin[:, 64:], -sin[:, :64])
        return np.concatenate([s_[:, HD // 2:], -s_[:, :HD // 2]], axis=1)

    bf = ml_dtypes.bfloat16
    wq = np.ascontiguousarray(np.asarray(Wq, dtype=np.float32).astype(bf))
    wk = np.ascontiguousarray(np.asarray(Wk, dtype=np.float32).astype(bf))
    wv = np.ascontiguousarray(np.asarray(Wv, dtype=np.float32).astype(bf))
    wo = np.ascontiguousarray(np.asarray(Wo, dtype=np.float32).astype(bf))

    in_maps = []
    for i in range(NC):
        b, pos = i // 4, i % 4
        js = [4 * s + 3 - pos for s in range(NSLOT)]
        take = lambda a: np.ascontiguousarray(
            np.concatenate([a[j * P:(j + 1) * P] for j in js], axis=0))
        m_tiles = [mask[js[s] * P:(js[s] + 1) * P, kb * P:(kb + 1) * P]
                   for (s, kb) in P_list]
        if not m_tiles:
            m_tiles.append(np.zeros((P, P), np.float32))
        in_maps.append({
            "x": take(hs[b]),
            "wq": wq, "wk": wk, "wv": wv, "wo": wo,
            "cosq": take(cos2 * scale),
            "sinq3": take(t3(sin2 * scale)),
            "cosk": take(cos2),
            "sink3": take(t3(sin2)),
            "masks": np.stack(m_tiles).astype(bf),
        })
    return in_maps


_cache = {}


def kernel(hidden_states, attention_mask, cos, sin, Wq, Wk, Wv, Wo,
           _trace=False, _trace_kwargs=None):
    from concourse.bass_utils import run_bass_kernel_spmd

    E, P_list = _mask_plan(attention_mask)
    key = (tuple(E), tuple(P_list))
    if key not in _cache:
        _cache[key] = _build_program(E, P_list)
    nc = _cache[key]

    in_maps = _prep_inputs(hidden_states, attention_mask, cos, sin,
                           Wq, Wk, Wv, Wo, P_list)
    kwargs = dict(_trace_kwargs or {})
    if _trace:
        kwargs["trace"] = True
    res = run_bass_kernel_spmd(nc, in_maps, list(range(NC)), **kwargs)

    out = np.empty((B, T, HID), dtype=np.float32)
    for i in range(NC):
        b, pos = i // 4, i % 4
        o = res.results[i]["out"]
        for s in range(NSLOT):
            j = 4 * s + 3 - pos
            out[b, j * P:(j + 1) * P, :] = o[s * P:(s + 1) * P, :]
    kernel._last_result = res
    return out
